# revision 10
# baseline (speedup 1.0000x reference)
"""Trainium2 Bass kernel for nn_Cell2Tissue (scatter_memory).

Reference computation:
  avg = AvgPool4x4(Conv3x3_SAME(cell) + bias)          # (128, 64, 64)
  for each tissue sample j: ROI_j += avg               # 64x64 ROI from loc
  output = stack of B copies of the mutated tissue     # (4, 4, 128, 256, 256)

Sharding over 8 cores: core c = (sample j = c % 4, channel half h = c // 4).
Each core computes avg for its 64 output channels (conv replicated, input
bf16), streams its tissue half through SBUF to the output, and overwrites
the dynamic ROI (offsets loaded from data into registers) with
tissue_roi + avg. The x4 output stack is a zero-copy host broadcast at
unshard time.

Conv: AvgPool4(Conv3x3(x)) == 6x6-tap stride-4 correlation with the 3x3
kernel pre-convolved with a 4x4/16 box (folded on host into the weights).
The stride-4 gather is done on host as a 16-plane polyphase split of the
zero-padded cell map, so every tensor-engine moving operand is stride-1
within rows (measured 533 -> ~300 ns per N=512 matmul). All 16 planes sit
resident in SBUF; taps are processed phase-major so matmuls chase the
plane DMAs; the 8 row-blocks accumulate in all 8 PSUM banks so each tap's
weights serve 8 matmuls.
"""

import os
import numpy as np

B, C, H, W = 4, 128, 256, 256
CH = C // 2          # channels per core (half)
L = 32               # half ROI width
ROI = 2 * L          # 64
NCORES = 8
PR = 66              # polyphase plane rows/cols (ceil(258/4)+1)
PHASES = 16

_CACHE = {}


def _get_modules():
    if "mods" in _CACHE:
        return _CACHE["mods"]
    # bass2jax executes via the jax 'axon'/'neuron' platform; a cpu-pinned
    # JAX_PLATFORMS would hide the devices.
    if os.environ.get("JAX_PLATFORMS") in ("cpu",):
        del os.environ["JAX_PLATFORMS"]
    import concourse.bass as bass
    import concourse.mybir as mybir
    import concourse.tile as tile
    from concourse.bass_utils import run_bass_kernel_spmd

    _CACHE["mods"] = (bass, mybir, tile, run_bass_kernel_spmd)
    return _CACHE["mods"]


def _split_multiwaits(nc, mybir, max_waits=1):
    """The walrus build here rejects >1 sem-wait on some instructions (the
    Tile tail InstDrain). Hoist extra waits onto single-wait nops placed
    immediately before, on the same engine (same-engine program order
    preserves semantics)."""
    for fn in nc.m.functions:
        for bb in fn.blocks:
            insts = bb.instructions
            i = 0
            while i < len(insts):
                inst = insts[i]
                si = inst.sync_info
                if si is not None and si.on_wait and len(si.on_wait) > max_waits:
                    waits = list(si.on_wait)
                    keep = waits[-max_waits:]
                    for k, w in enumerate(waits[:-max_waits]):
                        nop = mybir.InstNoOp(
                            name=f"{inst.name}_hoistwait_{k}",
                            sync_info=mybir.SyncInfo(on_wait=[w], on_update=[]),
                            bass_nofuse=True,
                            engine=inst.engine,
                        )
                        insts.insert(i, nop)
                        i += 1
                    si.on_wait = keep
                i += 1


def _build_program():
    """One SPMD program: per-core inputs
      tissue (64,256,256) f32, cell (128,16,66,66) bf16 polyphase (replicated),
      w6t (36,128,64) bf16, bias (64,1) f32, roff (1,2) i32 = [row0, col0]
    output: out (64,256,256) f32 = tissue with avg added in the ROI."""
    if "nc" in _CACHE:
        return _CACHE["nc"]
    bass, mybir, tile, _ = _get_modules()
    f32, bf16, i32 = mybir.dt.float32, mybir.dt.bfloat16, mybir.dt.int32

    nc = bass.Bass("TRN2", target_bir_lowering=False, debug=False,
                   num_devices=NCORES)
    tissue_d = nc.dram_tensor("tissue", (CH, H, W), f32, kind="ExternalInput").ap()
    cell_d = nc.dram_tensor("cell", (C, PHASES, PR, PR), bf16,
                            kind="ExternalInput").ap()
    w6t_d = nc.dram_tensor("w6t", (36, C, CH), bf16, kind="ExternalInput").ap()
    bias_d = nc.dram_tensor("bias", (CH, 1), f32, kind="ExternalInput").ap()
    roff_d = nc.dram_tensor("roff", (1, 2), i32, kind="ExternalInput").ap()
    out_d = nc.dram_tensor("out", (CH, H, W), f32, kind="ExternalOutput").ap()

    NBLK = 8             # conv row blocks: 8 output rows, one PSUM bank each
    ORB = ROI // NBLK    # 8 output rows per block

    # taps grouped by polyphase plane so matmuls chase the plane DMAs
    tap_order = []       # (tap_idx, plane, row_shift, col_shift)
    for pp in range(4):
        for qq in range(4):
            for p in range(pp, 6, 4):
                for q in range(qq, 6, 4):
                    tap_order.append((p * 6 + q, pp * 4 + qq, p // 4, q // 4))
    assert len(tap_order) == 36

    with tile.TileContext(nc) as tc:
        with (
            tc.tile_pool(name="const", bufs=1) as constp,
            tc.tile_pool(name="cellp", bufs=1) as cellp,
            tc.tile_pool(name="roip", bufs=1) as roip,
            tc.tile_pool(name="copyp", bufs=3) as copyp,
            tc.tile_pool(name="psum", bufs=1, space="PSUM") as psump,
        ):
            # --- constants (scalar-engine DGE queue; copies own sync's) ---
            w_sb = constp.tile([C, 36 * CH], bf16)
            # w6t (36, C, CH) -> partition=input channel, free=(tap, out ch)
            nc.scalar.dma_start(w_sb[:], w6t_d.rearrange("t i o -> i t o"))
            bias_sb = constp.tile([CH, 1], f32)
            nc.scalar.dma_start(bias_sb[:], bias_d[:])
            roff_sb = constp.tile([1, 2], i32)
            nc.scalar.dma_start(roff_sb[:], roff_d[:])

            # offsets are in-bounds by construction; the runtime assert's
            # ISA op miscompiles on this walrus build
            r_v = nc.values_load(roff_sb[0:1, 0:1], min_val=0, max_val=H - ROI,
                                 skip_runtime_bounds_check=True)
            c_v = nc.values_load(roff_sb[0:1, 1:2], min_val=0, max_val=W - ROI,
                                 skip_runtime_bounds_check=True)

            # ROI source pixels: load early, overlaps with everything below
            roi_sb = roip.tile([CH, ROI * ROI], f32)
            nc.scalar.dma_start(
                roi_sb[:], tissue_d[:, bass.ds(r_v, ROI), bass.ds(c_v, ROI)]
            )

            # --- bulk copy tissue -> out ---
            # view (64, 256, 256) as (half s, row p=128, chan c, col w)
            KC = 8  # channels per copy tile
            t_v = tissue_d.rearrange("c (s p) w -> s p c w", p=128)
            o_v = out_d.rearrange("c (s p) w -> s p c w", p=128)
            for s in range(2):
                for cg in range(CH // KC):
                    ct = copyp.tile([128, KC * W], f32, tag="cp")
                    nc.sync.dma_start(ct[:], t_v[s, :, cg * KC:(cg + 1) * KC, :])
                    nc.sync.dma_start(o_v[s, :, cg * KC:(cg + 1) * KC, :], ct[:])

            # --- polyphase cell planes, resident in SBUF ---
            cell_t = cellp.tile([C, PHASES * PR * PR], bf16)
            c4 = cell_t.rearrange("c (ph r w) -> c ph r w", r=PR, w=PR)
            for ph in range(PHASES):
                nc.scalar.dma_start(c4[:, ph], cell_d[:, ph])

            # --- conv: 36 taps x 8 blocks, accumulating in 8 PSUM banks ---
            pss = [psump.tile([CH, ORB * ROI], f32, name=f"bank{b}")
                   for b in range(NBLK)]
            for i, (t, ph, pb, qb) in enumerate(tap_order):
                for b in range(NBLK):
                    nc.tensor.matmul(
                        pss[b][:],
                        w_sb[:, t * CH:(t + 1) * CH],
                        c4[:, ph, b * ORB + pb:b * ORB + pb + ORB, qb:qb + ROI],
                        start=(i == 0),
                        stop=(i == 35),
                    )
            # roi strip b += psum[b] + bias  (fused on DVE)
            for b in range(NBLK):
                strip = slice(b * ORB * ROI, (b + 1) * ORB * ROI)
                nc.vector.scalar_tensor_tensor(
                    roi_sb[:, strip], pss[b][:], bias_sb[:], roi_sb[:, strip],
                    mybir.AluOpType.add, mybir.AluOpType.add,
                )

            # --- ROI scatter: overwrite after all bulk writes landed ---
            tc.strict_bb_all_engine_barrier()
            nc.sync.dma_start(
                out_d[:, bass.ds(r_v, ROI), bass.ds(c_v, ROI)],
                roi_sb[:],
            )

    _split_multiwaits(nc, mybir)
    _CACHE["nc"] = nc
    return nc


def _prep_inputs(tissue_features, cell_features, loc, conv_w, conv_b):
    import ml_dtypes

    bf16 = ml_dtypes.bfloat16
    # fold AvgPool4x4 into the conv kernel: 6x6 taps
    w6 = np.zeros((C, C, 6, 6), np.float32)
    for dr in range(4):
        for dc in range(4):
            w6[:, :, dr:dr + 3, dc:dc + 3] += conv_w
    w6 *= 1.0 / 16.0

    # polyphase split of the zero-padded cell map:
    # plane (pp,qq)[y,x] = padded[4y+pp, 4x+qq], padded = 1px zero border
    padc = np.zeros((C, 4 * PR, 4 * PR), np.float32)
    padc[:, 1:1 + H, 1:1 + W] = cell_features[0]
    cell_poly = np.empty((C, PHASES, PR, PR), np.float32)
    for pp in range(4):
        for qq in range(4):
            cell_poly[:, pp * 4 + qq] = padc[:, pp::4, qq::4]
    cell_poly = np.ascontiguousarray(cell_poly).astype(bf16)

    w6t = {}
    bias = {}
    for h in range(2):
        sl = slice(CH * h, CH * (h + 1))
        # (CH, C, 6, 6) -> (tap, in ch, out ch)
        w6t[h] = np.ascontiguousarray(
            w6[sl].transpose(2, 3, 1, 0).reshape(36, C, CH)
        ).astype(bf16)
        bias[h] = np.ascontiguousarray(conv_b[sl].astype(np.float32)).reshape(CH, 1)

    r0 = loc[:, 1].astype(np.int64) * W // 1024 - L   # H-dim start (from loc x)
    c0 = loc[:, 0].astype(np.int64) * W // 1024 - L   # W-dim start (from loc y)

    in_maps = []
    for c in range(NCORES):
        j, h = c % B, c // B
        in_maps.append({
            "tissue": tissue_features[j, CH * h:CH * (h + 1)],
            "cell": cell_poly,
            "w6t": w6t[h],
            "bias": bias[h],
            "roff": np.array([[r0[j], c0[j]]], np.int32),
        })
    return in_maps


def run_device(tissue_features, cell_features, loc, conv_w, conv_b, **spmd_kwargs):
    """Build+run the SPMD kernel; returns (final (4,128,256,256), raw results)."""
    *_, run_bass_kernel_spmd = _get_modules()
    nc = _build_program()
    in_maps = _prep_inputs(tissue_features, cell_features, loc, conv_w, conv_b)
    res = run_bass_kernel_spmd(nc, in_maps, list(range(NCORES)), **spmd_kwargs)
    final = np.empty((B, C, H, W), np.float32)
    for c in range(NCORES):
        j, h = c % B, c // B
        final[j, CH * h:CH * (h + 1)] = res.results[c]["out"]
    return final, res


def kernel(tissue_features, cell_features, loc, conv_w, conv_b):
    final, _ = run_device(tissue_features, cell_features, loc, conv_w, conv_b)
    # reference stacks B copies of the fully-mutated tissue
    return np.broadcast_to(final[None], (B, B, C, H, W))
